# revision 13
# baseline (speedup 1.0000x reference)
"""CRF negative-log-likelihood kernel for Trainium2 (8 NeuronCores).

Math: reference computes  partition - gold  where
  partition = sum_b logsumexp_c(alpha[511])  via the forward algorithm
  gold      = sum emissions[b,s,tags] * m + sum T[tags[s],tags[s+1]] * m[:,1:]

Device strategy (data-parallel over batch, 32 rows per core):
  * Linear domain: alpha_t = E_t o (A' @ alpha_{t-1}) with A' = exp(T) e^-g,
    E_t = exp(e_t).  The per-step logsumexp becomes a [128,128]x[128,32]
    matmul (PE) plus an elementwise multiply (DVE).
  * Bidirectional scan, PAIRED: forward (A'^T) and backward (A') states
    live in one [128,64] tile [alphaF | vB]; the host lays emissions out
    so pair-step k holds [E_k | E_{511-k}].  Per step: 2 matmuls into one
    PSUM tile + ONE DVE multiply.  256 serial steps instead of 511.
  * Stability WITHOUT renorm: the constant growth rate g=GAMMA (calibrated
    offline; per-step ln growth of the scan state) is folded into the
    transition matrix on the host.  State magnitudes then do a bounded
    random walk (~2^+-14, far inside bf16 range) and the host adds the
    exact known correction 511*GAMMA per batch element.  This keeps the
    scan loop free of colsum/reciprocal/broadcast work, so PE and DVE run
    nothing but the serial chain.
  * Gold emit: masked sum e o onehot(tags) chunk-wise entirely on GPSIMD
    (Pool) via fused scalar_tensor_tensor with accum_out; Pool is off the
    scan critical path, so no anchoring is needed.
  * Gold trans: exact masked pair-count matrix CNT (host-built, index-only
    preprocessing) dotted with T on Pool at the end.
Outputs per core: meeting-dot row pdrow, gold partials; host sums in
float64, adds 511*GAMMA per batch element, returns a float32 scalar.
"""

import sys

for _p in ("/opt/trn_rl_repo",):
    if _p not in sys.path:
        sys.path.insert(0, _p)

import os as _os
import numpy as np
import ml_dtypes
from contextlib import ExitStack

from concourse import bass, tile, mybir, bacc
from concourse.bass_utils import run_bass_kernel_spmd

NCORES = 8
B, S, C = 256, 512, 128
BC = B // NCORES          # batch rows per core
FREE = S * BC             # free-dim elements of the per-core emission tensor
PAIRW = 2 * BC            # 64: [E_k | E_{S-1-k}]
HALF = S // 2             # pair-steps: fwd e_0..e_255, bwd e_256..e_511

# calibrated mean per-step ln growth of the paired scan state (see
# calibrate.py); folded into the transition matrix as exp(-GAMMA) and
# compensated exactly on the host with +511*GAMMA per batch element.
GAMMA = 5.8644

# emission chunk sizes (free elements); small leading chunks let the scan
# chain start before the bulk DMA+exp completes
CH_SIZES = [64, 192, 768, 1024] + [2048] * 7
CH_OFF = [0]
for _s in CH_SIZES:
    CH_OFF.append(CH_OFF[-1] + _s)
assert CH_OFF[-1] == FREE
NCHUNK = len(CH_SIZES)

F32 = mybir.dt.float32
BF16 = mybir.dt.bfloat16
AF = mybir.ActivationFunctionType
OP = mybir.AluOpType

_EN_GOLD = _os.environ.get("CRF_GOLD", "1") == "1"
_EN_SCAN = _os.environ.get("CRF_SCAN", "1") == "1"

_NC_CACHE = None


def _build_nc():
    nc = bacc.Bacc("TRN2", target_bir_lowering=False, debug=False)

    et = nc.dram_tensor("et", [C, FREE], BF16, kind="ExternalInput").ap()
    afwd = nc.dram_tensor("afwd", [C, C], BF16, kind="ExternalInput").ap()
    abwd = nc.dram_tensor("abwd", [C, C], BF16, kind="ExternalInput").ap()
    hemit = nc.dram_tensor("hemit", [C, FREE], BF16, kind="ExternalInput").ap()
    cnt_in = nc.dram_tensor("cnt", [C, C], F32, kind="ExternalInput").ap()
    tsb_in = nc.dram_tensor("tsb", [C, C], F32, kind="ExternalInput").ap()
    pdrow = nc.dram_tensor("pdrow", [C, BC], F32, kind="ExternalOutput").ap()
    gold = nc.dram_tensor("gold", [128, 1], F32, kind="ExternalOutput").ap()

    with tile.TileContext(nc) as tc, ExitStack() as ctx:
        sb = ctx.enter_context(tc.tile_pool(name="sb", bufs=1))
        wk = ctx.enter_context(tc.tile_pool(name="wk", bufs=4))
        ps = ctx.enter_context(tc.tile_pool(name="ps", bufs=2, space="PSUM"))

        # ---- persistent tiles; chunk0 DMA first (longest startup path) --
        wf = sb.tile([C, C], BF16, name="wf")
        wb_ = sb.tile([C, C], BF16, name="wb")

        # ---- emission chunks: DMA in + exp ------------------------------
        raws, ecs = [], []
        et_dmas = []
        for k, csz in enumerate(CH_SIZES):
            raw = sb.tile([C, csz], BF16, name=f"raw{k}")
            et_dmas.append(
                nc.sync.dma_start(raw[:], et[:, CH_OFF[k]:CH_OFF[k] + csz]))
            raws.append(raw)
            ec = sb.tile([C, csz], BF16, name=f"ec{k}")
            ecs.append(ec)
            if k == 0:
                nc.sync.dma_start(wf[:], afwd[:])
                nc.sync.dma_start(wb_[:], abwd[:])

        NEARLY = 3            # chunks whose exp runs before the scan starts
        def exp_chunk(c):
            nc.scalar.activation(ecs[c][:], raws[c][:], AF.Exp)
        for c in range(NEARLY):
            exp_chunk(c)

        def ec_pair(k, lo=0, hi=PAIRW):
            pos = k * PAIRW
            for c in range(NCHUNK):
                if pos < CH_OFF[c + 1]:
                    o = pos - CH_OFF[c]
                    return ecs[c][:, o + lo:o + hi]
            raise IndexError(k)

        # ---- gold: all on Pool (off the scan critical path) -------------
        from concourse.tile_rust import add_dep_helper
        gold_finish = None
        if not _EN_GOLD:
            zg = sb.tile([128, 1], F32, name="zg")
            nc.vector.memset(zg[:], 0.0)
            nc.sync.dma_start(gold[:], zg[:])
        if not _EN_SCAN:
            zl = sb.tile([C, BC], F32, name="zl")
            nc.vector.memset(zl[:], 1.0)
            nc.sync.dma_start(pdrow[:], zl[:])

        if _EN_GOLD:
            hem_sb = sb.tile([C, FREE], BF16, name="hem_sb")
            cnt_sb = sb.tile([C, C], F32, name="cnt_sb")
            tsb = sb.tile([C, C], F32, name="tsb_t")
            last_et = et_dmas[-1].ins
            qs = FREE // 8
            for k in range(8):
                gd = nc.sync.dma_start(hem_sb[:, k * qs:(k + 1) * qs],
                                       hemit[:, k * qs:(k + 1) * qs])
                add_dep_helper(gd.ins, last_et,
                               reason="gold DMA after emission stream")
            for gd in (nc.sync.dma_start(cnt_sb[:], cnt_in[:]),
                       nc.sync.dma_start(tsb[:], tsb_in[:])):
                add_dep_helper(gd.ins, last_et,
                               reason="gold DMA after emission stream")

            gold_acc = sb.tile([128, 1], F32, name="gold_acc")
            nc.gpsimd.memset(gold_acc[:], 0.0)

            # emit pieces: fused multiply+row-sum on Pool
            pieces = []
            for c, csz in enumerate(CH_SIZES):
                o = 0
                while o < csz:
                    w = min(512, csz - o)
                    pieces.append((c, o, w))
                    o += w

            def emit_piece(j):
                c, o, w = pieces[j]
                scratch = wk.tile([C, 512], BF16, tag="scr", bufs=2,
                                  name=f"scr{j}")
                epk = wk.tile([128, 1], F32, tag="ep", bufs=2, name=f"ep{j}")
                nc.gpsimd.tensor_mul(
                    scratch[:, 0:w], raws[c][:, o:o + w],
                    hem_sb[:, CH_OFF[c] + o:CH_OFF[c] + o + w])
                nc.scalar.activation(scratch[:, 0:w], scratch[:, 0:w],
                                     AF.Identity, accum_out=epk[:])
                nc.gpsimd.tensor_add(gold_acc[:], gold_acc[:], epk[:])

            for j in range(len(pieces)):
                emit_piece(j)

            def gold_finish():
                trash = sb.tile([128, 128], F32, name="trash")
                tp = sb.tile([128, 1], F32, name="tp")
                nc.gpsimd.tensor_mul(trash[:], cnt_sb[:], tsb[:])
                nc.scalar.activation(trash[:], trash[:], AF.Identity,
                                     accum_out=tp[:])
                gold_sb = sb.tile([128, 1], F32, name="gold_sb")
                nc.gpsimd.tensor_add(gold_sb[:], gold_acc[:], tp[:])
                nc.sync.dma_start(gold[:], gold_sb[:])

        # injection schedule: value = list of zero-arg callables issued
        # after scan step k (controls Activation-queue ordering only)
        inject_at = {}
        if _EN_SCAN:
            for c in range(NEARLY, NCHUNK):
                k_need = CH_OFF[c] // PAIRW
                lead = 8 if c < 4 else 20
                inject_at.setdefault(max(2, k_need - lead), []).append(
                    lambda c=c: exp_chunk(c))
        else:
            for c in range(NEARLY, NCHUNK):
                exp_chunk(c)

        if _EN_SCAN:
            # ---- bidirectional scan, SPLIT chains: the forward and ------
            # backward states are independent serial chains (they only meet
            # at the end), so each runs its own MM -> TT cycle; the two
            # cycles interleave on PE/DVE, halving per-instruction exec on
            # the critical path.
            aF = ec_pair(0, 0, BC)          # E_0
            vB = ec_pair(0, BC, PAIRW)      # E_511
            for k in range(1, HALF):
                ppF = ps.tile([C, BC], F32, tag="ppF", bufs=3, name=f"pf{k}")
                nc.tensor.matmul(ppF[:], wf[:], aF, start=True, stop=True)
                aF_new = wk.tile([C, BC], BF16, tag="aF", bufs=6,
                                 name=f"aF{k}")
                nc.vector.tensor_tensor(aF_new[:], ppF[:], ec_pair(k, 0, BC),
                                        op=OP.mult)
                aF = aF_new[:]

                ppB = ps.tile([C, BC], F32, tag="ppB", bufs=3, name=f"pb{k}")
                nc.tensor.matmul(ppB[:], wb_[:], vB, start=True, stop=True)
                vB_new = wk.tile([C, BC], BF16, tag="vB", bufs=6,
                                 name=f"vB{k}")
                nc.vector.tensor_tensor(vB_new[:], ppB[:],
                                        ec_pair(k, BC, PAIRW), op=OP.mult)
                vB = vB_new[:]
                for job in inject_at.get(k, []):
                    job()

            # ---- combine: d[c,b] = alphaF[c,b] * (A' vB)[c,b]; the ------
            # column sum and log run on the host (shorter device tail)
            pbf = ps.tile([C, BC], F32, tag="ppB", bufs=3, name="pb_final")
            nc.tensor.matmul(pbf[:], wb_[:], vB, start=True, stop=True)
            d = wk.tile([C, BC], F32, tag="dm", bufs=1, name="d_meet")
            nc.vector.tensor_tensor(d[:], pbf[:], aF, op=OP.mult)
            nc.sync.dma_start(pdrow[:], d[:])
        if _EN_GOLD:
            gold_finish()

    nc.compile()
    return nc


def _prep_inputs(emissions, tags, mask, transitions):
    em = np.asarray(emissions, dtype=np.float32)
    tg = np.asarray(tags).astype(np.int64)
    mk = np.asarray(mask).astype(np.float32)
    tr = np.ascontiguousarray(np.asarray(transitions, dtype=np.float32))

    a_f = np.exp(tr.astype(np.float64) - GAMMA)
    afwd = a_f.astype(ml_dtypes.bfloat16)
    abwd = np.ascontiguousarray(a_f.T).astype(ml_dtypes.bfloat16)

    # paired free layout: pair-step k holds [E_k | E_{S-1-k}] in 64 cols
    s_all = np.arange(S, dtype=np.int64)
    pair_base = np.where(s_all < S // 2, s_all * PAIRW,
                         (S - 1 - s_all) * PAIRW + BC)   # [S]
    b_rows = np.arange(BC, dtype=np.int64)[:, None]      # [BC,1]
    sbcol = (pair_base[None, :] + b_rows).ravel()        # free idx for (b,s)

    in_maps = []
    for core in range(NCORES):
        b0 = core * BC
        ec = em[b0:b0 + BC]                              # [BC,S,C]
        ett = ec.transpose(2, 1, 0)                      # [C,S,BC]
        half = S // 2
        et = np.empty((C, half, PAIRW), dtype=np.float32)
        et[:, :, :BC] = ett[:, :half, :]                 # fwd slot: E_k
        et[:, :, BC:] = ett[:, :half - 1:-1, :]          # bwd slot: E_{S-1-k}
        et = np.ascontiguousarray(
            et.reshape(C, FREE)).astype(ml_dtypes.bfloat16)

        tgc = tg[b0:b0 + BC]                             # [BC,S]
        mkc = mk[b0:b0 + BC]

        hemit = np.zeros((C, FREE), dtype=ml_dtypes.bfloat16)
        hemit[tgc.ravel(), sbcol] = mkc.ravel()

        # masked pair-count histogram (index-only preprocessing; the
        # float gather-sum  sum T[i,j]*CNT[i,j]  runs on device)
        cnt = np.zeros((C, C), dtype=np.float64)
        np.add.at(cnt, (tgc[:, :-1].ravel(), tgc[:, 1:].ravel()),
                  mkc[:, 1:].ravel().astype(np.float64))
        cnt = cnt.astype(np.float32)

        in_maps.append({
            "et": et, "afwd": afwd, "abwd": abwd,
            "hemit": hemit, "cnt": cnt, "tsb": tr,
        })
    return in_maps


def kernel(emissions, tags, mask, transitions, _trace=False):
    global _NC_CACHE
    if _NC_CACHE is None:
        _NC_CACHE = _build_nc()
    nc = _NC_CACHE

    in_maps = _prep_inputs(emissions, tags, mask, transitions)
    res = run_bass_kernel_spmd(
        nc, in_maps, core_ids=list(range(NCORES)), trace=_trace,
    )
    partition = np.float64(0.0)
    gold = np.float64(0.0)
    for r in res.results:
        pd = np.asarray(r["pdrow"], dtype=np.float64).sum(axis=0)
        partition += (np.log(pd) + 511.0 * GAMMA).sum()
        gold += np.asarray(r["gold"], dtype=np.float64).sum()
    out = np.float32(partition - gold)
    if _trace:
        return out, res
    return out


# revision 14
# speedup vs baseline: 1.0108x; 1.0108x over previous
"""CRF negative-log-likelihood kernel for Trainium2 (8 NeuronCores).

Math: reference computes  partition - gold  where
  partition = sum_b logsumexp_c(alpha[511])  via the forward algorithm
  gold      = sum emissions[b,s,tags] * m + sum T[tags[s],tags[s+1]] * m[:,1:]

Device strategy (data-parallel over batch, 32 rows per core):
  * Linear domain: alpha_t = E_t o (A' @ alpha_{t-1}) with A' = exp(T) e^-g,
    E_t = exp(e_t).  The per-step logsumexp becomes a [128,128]x[128,32]
    matmul (PE) plus an elementwise multiply (DVE).
  * Bidirectional scan, PAIRED: forward (A'^T) and backward (A') states
    live in one [128,64] tile [alphaF | vB]; the host lays emissions out
    so pair-step k holds [E_k | E_{511-k}].  Per step: 2 matmuls into one
    PSUM tile + ONE DVE multiply.  256 serial steps instead of 511.
  * Stability WITHOUT renorm: the constant growth rate g=GAMMA (calibrated
    offline; per-step ln growth of the scan state) is folded into the
    transition matrix on the host.  State magnitudes then do a bounded
    random walk (~2^+-14, far inside bf16 range) and the host adds the
    exact known correction 511*GAMMA per batch element.  This keeps the
    scan loop free of colsum/reciprocal/broadcast work, so PE and DVE run
    nothing but the serial chain.
  * Gold emit: masked sum e o onehot(tags) chunk-wise entirely on GPSIMD
    (Pool) via fused scalar_tensor_tensor with accum_out; Pool is off the
    scan critical path, so no anchoring is needed.
  * Gold trans: exact masked pair-count matrix CNT (host-built, index-only
    preprocessing) dotted with T on Pool at the end.
Outputs per core: meeting-dot row pdrow, gold partials; host sums in
float64, adds 511*GAMMA per batch element, returns a float32 scalar.
"""

import sys

for _p in ("/opt/trn_rl_repo",):
    if _p not in sys.path:
        sys.path.insert(0, _p)

import os as _os
import numpy as np
import ml_dtypes
from contextlib import ExitStack

from concourse import bass, tile, mybir, bacc
from concourse.bass_utils import run_bass_kernel_spmd

NCORES = 8
B, S, C = 256, 512, 128
BC = B // NCORES          # batch rows per core
FREE = S * BC             # free-dim elements of the per-core emission tensor
PAIRW = 2 * BC            # 64: [E_k | E_{S-1-k}]
HALF = S // 2             # pair-steps: fwd e_0..e_255, bwd e_256..e_511

# calibrated mean per-step ln growth of the paired scan state (see
# calibrate.py); folded into the transition matrix as exp(-GAMMA) and
# compensated exactly on the host with +511*GAMMA per batch element.
GAMMA = 5.8644

# emission chunk sizes (free elements); small leading chunks let the scan
# chain start before the bulk DMA+exp completes
CH_SIZES = [320, 704, 1024] + [2048] * 7
CH_OFF = [0]
for _s in CH_SIZES:
    CH_OFF.append(CH_OFF[-1] + _s)
assert CH_OFF[-1] == FREE
NCHUNK = len(CH_SIZES)

F32 = mybir.dt.float32
BF16 = mybir.dt.bfloat16
AF = mybir.ActivationFunctionType
OP = mybir.AluOpType

_EN_GOLD = _os.environ.get("CRF_GOLD", "1") == "1"
_EN_SCAN = _os.environ.get("CRF_SCAN", "1") == "1"

_NC_CACHE = None


def _build_nc():
    nc = bacc.Bacc("TRN2", target_bir_lowering=False, debug=False)

    et = nc.dram_tensor("et", [C, FREE], BF16, kind="ExternalInput").ap()
    afwd = nc.dram_tensor("afwd", [C, C], BF16, kind="ExternalInput").ap()
    abwd = nc.dram_tensor("abwd", [C, C], BF16, kind="ExternalInput").ap()
    hemit = nc.dram_tensor("hemit", [C, FREE], BF16, kind="ExternalInput").ap()
    cnt_in = nc.dram_tensor("cnt", [C, C], F32, kind="ExternalInput").ap()
    tsb_in = nc.dram_tensor("tsb", [C, C], F32, kind="ExternalInput").ap()
    pdrow = nc.dram_tensor("pdrow", [C, BC], F32, kind="ExternalOutput").ap()
    gold = nc.dram_tensor("gold", [128, 1], F32, kind="ExternalOutput").ap()

    with tile.TileContext(nc) as tc, ExitStack() as ctx:
        sb = ctx.enter_context(tc.tile_pool(name="sb", bufs=1))
        wk = ctx.enter_context(tc.tile_pool(name="wk", bufs=4))
        ps = ctx.enter_context(tc.tile_pool(name="ps", bufs=2, space="PSUM"))

        # ---- persistent tiles; chunk0 DMA first (longest startup path) --
        wf = sb.tile([C, C], BF16, name="wf")
        wb_ = sb.tile([C, C], BF16, name="wb")

        # ---- emission chunks: DMA in + exp ------------------------------
        raws, ecs = [], []
        et_dmas = []
        for k, csz in enumerate(CH_SIZES):
            raw = sb.tile([C, csz], BF16, name=f"raw{k}")
            et_dmas.append(
                nc.sync.dma_start(raw[:], et[:, CH_OFF[k]:CH_OFF[k] + csz]))
            raws.append(raw)
            ec = sb.tile([C, csz], BF16, name=f"ec{k}")
            ecs.append(ec)
            if k == 0:
                nc.sync.dma_start(wf[:], afwd[:])
                nc.sync.dma_start(wb_[:], abwd[:])

        NEARLY = 2            # chunks whose exp runs before the scan starts
        def exp_chunk(c):
            nc.scalar.activation(ecs[c][:], raws[c][:], AF.Exp)
        for c in range(NEARLY):
            exp_chunk(c)

        def ec_pair(k, lo=0, hi=PAIRW):
            pos = k * PAIRW
            for c in range(NCHUNK):
                if pos < CH_OFF[c + 1]:
                    o = pos - CH_OFF[c]
                    return ecs[c][:, o + lo:o + hi]
            raise IndexError(k)

        # ---- gold: all on Pool (off the scan critical path) -------------
        from concourse.tile_rust import add_dep_helper
        gold_finish = None
        if not _EN_GOLD:
            zg = sb.tile([128, 1], F32, name="zg")
            nc.vector.memset(zg[:], 0.0)
            nc.sync.dma_start(gold[:], zg[:])
        if not _EN_SCAN:
            zl = sb.tile([C, BC], F32, name="zl")
            nc.vector.memset(zl[:], 1.0)
            nc.sync.dma_start(pdrow[:], zl[:])

        if _EN_GOLD:
            hem_sb = sb.tile([C, FREE], BF16, name="hem_sb")
            cnt_sb = sb.tile([C, C], F32, name="cnt_sb")
            tsb = sb.tile([C, C], F32, name="tsb_t")
            last_et = et_dmas[-1].ins
            qs = FREE // 8
            for k in range(8):
                gd = nc.sync.dma_start(hem_sb[:, k * qs:(k + 1) * qs],
                                       hemit[:, k * qs:(k + 1) * qs])
                add_dep_helper(gd.ins, last_et,
                               reason="gold DMA after emission stream")
            for gd in (nc.sync.dma_start(cnt_sb[:], cnt_in[:]),
                       nc.sync.dma_start(tsb[:], tsb_in[:])):
                add_dep_helper(gd.ins, last_et,
                               reason="gold DMA after emission stream")

            gold_acc = sb.tile([128, 1], F32, name="gold_acc")
            nc.gpsimd.memset(gold_acc[:], 0.0)

            # emit pieces: fused multiply+row-sum on Pool
            pieces = []
            for c, csz in enumerate(CH_SIZES):
                o = 0
                while o < csz:
                    w = min(512, csz - o)
                    pieces.append((c, o, w))
                    o += w

            def emit_piece(j):
                c, o, w = pieces[j]
                scratch = wk.tile([C, 512], BF16, tag="scr", bufs=2,
                                  name=f"scr{j}")
                epk = wk.tile([128, 1], F32, tag="ep", bufs=2, name=f"ep{j}")
                nc.gpsimd.tensor_mul(
                    scratch[:, 0:w], raws[c][:, o:o + w],
                    hem_sb[:, CH_OFF[c] + o:CH_OFF[c] + o + w])
                nc.scalar.activation(scratch[:, 0:w], scratch[:, 0:w],
                                     AF.Identity, accum_out=epk[:])
                nc.gpsimd.tensor_add(gold_acc[:], gold_acc[:], epk[:])

            def gold_finish():
                trash = sb.tile([128, 128], F32, name="trash")
                tp = sb.tile([128, 1], F32, name="tp")
                nc.gpsimd.tensor_mul(trash[:], cnt_sb[:], tsb[:])
                nc.scalar.activation(trash[:], trash[:], AF.Identity,
                                     accum_out=tp[:])
                gold_sb = sb.tile([128, 1], F32, name="gold_sb")
                nc.gpsimd.tensor_add(gold_sb[:], gold_acc[:], tp[:])
                nc.sync.dma_start(gold[:], gold_sb[:])

        # injection schedule: value = list of zero-arg callables issued
        # after scan step k.  This controls per-engine FIFO order only:
        # exp chunks must reach the Activation queue ahead of emit-accum
        # pieces that block on the (late) hemit DMA stream.
        inject_at = {}
        if _EN_SCAN:
            for c in range(NEARLY, NCHUNK):
                k_need = CH_OFF[c] // PAIRW
                lead = 8 if c < 3 else 20
                inject_at.setdefault(max(2, k_need - lead), []).append(
                    lambda c=c: exp_chunk(c))
            if _EN_GOLD:
                for j in range(len(pieces)):
                    inject_at.setdefault(40 + 6 * j, []).append(
                        lambda j=j: emit_piece(j))
        else:
            for c in range(NEARLY, NCHUNK):
                exp_chunk(c)
            if _EN_GOLD:
                for j in range(len(pieces)):
                    emit_piece(j)

        if _EN_SCAN:
            # ---- bidirectional scan, SPLIT chains: the forward and ------
            # backward states are independent serial chains (they only meet
            # at the end), so each runs its own MM -> TT cycle; the two
            # cycles interleave on PE/DVE, halving per-instruction exec on
            # the critical path.
            aF = ec_pair(0, 0, BC)          # E_0
            vB = ec_pair(0, BC, PAIRW)      # E_511
            for k in range(1, HALF):
                ppF = ps.tile([C, BC], F32, tag="ppF", bufs=3, name=f"pf{k}")
                nc.tensor.matmul(ppF[:], wf[:], aF, start=True, stop=True)
                aF_new = wk.tile([C, BC], BF16, tag="aF", bufs=6,
                                 name=f"aF{k}")
                nc.vector.tensor_tensor(aF_new[:], ppF[:], ec_pair(k, 0, BC),
                                        op=OP.mult)
                aF = aF_new[:]

                ppB = ps.tile([C, BC], F32, tag="ppB", bufs=3, name=f"pb{k}")
                nc.tensor.matmul(ppB[:], wb_[:], vB, start=True, stop=True)
                vB_new = wk.tile([C, BC], BF16, tag="vB", bufs=6,
                                 name=f"vB{k}")
                nc.vector.tensor_tensor(vB_new[:], ppB[:],
                                        ec_pair(k, BC, PAIRW), op=OP.mult)
                vB = vB_new[:]
                for job in inject_at.get(k, []):
                    job()

            # ---- combine: d[c,b] = alphaF[c,b] * (A' vB)[c,b]; the ------
            # column sum and log run on the host (shorter device tail)
            pbf = ps.tile([C, BC], F32, tag="ppB", bufs=3, name="pb_final")
            nc.tensor.matmul(pbf[:], wb_[:], vB, start=True, stop=True)
            d = wk.tile([C, BC], F32, tag="dm", bufs=1, name="d_meet")
            nc.vector.tensor_tensor(d[:], pbf[:], aF, op=OP.mult)
            nc.sync.dma_start(pdrow[:], d[:])
        if _EN_GOLD:
            gold_finish()

    nc.compile()
    return nc


def _prep_inputs(emissions, tags, mask, transitions):
    em = np.asarray(emissions, dtype=np.float32)
    tg = np.asarray(tags).astype(np.int64)
    mk = np.asarray(mask).astype(np.float32)
    tr = np.ascontiguousarray(np.asarray(transitions, dtype=np.float32))

    a_f = np.exp(tr.astype(np.float64) - GAMMA)
    afwd = a_f.astype(ml_dtypes.bfloat16)
    abwd = np.ascontiguousarray(a_f.T).astype(ml_dtypes.bfloat16)

    # paired free layout: pair-step k holds [E_k | E_{S-1-k}] in 64 cols
    s_all = np.arange(S, dtype=np.int64)
    pair_base = np.where(s_all < S // 2, s_all * PAIRW,
                         (S - 1 - s_all) * PAIRW + BC)   # [S]
    b_rows = np.arange(BC, dtype=np.int64)[:, None]      # [BC,1]
    sbcol = (pair_base[None, :] + b_rows).ravel()        # free idx for (b,s)

    in_maps = []
    for core in range(NCORES):
        b0 = core * BC
        ec = em[b0:b0 + BC]                              # [BC,S,C]
        ett = ec.transpose(2, 1, 0)                      # [C,S,BC]
        half = S // 2
        et = np.empty((C, half, PAIRW), dtype=np.float32)
        et[:, :, :BC] = ett[:, :half, :]                 # fwd slot: E_k
        et[:, :, BC:] = ett[:, :half - 1:-1, :]          # bwd slot: E_{S-1-k}
        et = np.ascontiguousarray(
            et.reshape(C, FREE)).astype(ml_dtypes.bfloat16)

        tgc = tg[b0:b0 + BC]                             # [BC,S]
        mkc = mk[b0:b0 + BC]

        hemit = np.zeros((C, FREE), dtype=ml_dtypes.bfloat16)
        hemit[tgc.ravel(), sbcol] = mkc.ravel()

        # masked pair-count histogram (index-only preprocessing; the
        # float gather-sum  sum T[i,j]*CNT[i,j]  runs on device)
        cnt = np.zeros((C, C), dtype=np.float64)
        np.add.at(cnt, (tgc[:, :-1].ravel(), tgc[:, 1:].ravel()),
                  mkc[:, 1:].ravel().astype(np.float64))
        cnt = cnt.astype(np.float32)

        in_maps.append({
            "et": et, "afwd": afwd, "abwd": abwd,
            "hemit": hemit, "cnt": cnt, "tsb": tr,
        })
    return in_maps


def kernel(emissions, tags, mask, transitions, _trace=False):
    global _NC_CACHE
    if _NC_CACHE is None:
        _NC_CACHE = _build_nc()
    nc = _NC_CACHE

    in_maps = _prep_inputs(emissions, tags, mask, transitions)
    res = run_bass_kernel_spmd(
        nc, in_maps, core_ids=list(range(NCORES)), trace=_trace,
    )
    partition = np.float64(0.0)
    gold = np.float64(0.0)
    for r in res.results:
        pd = np.asarray(r["pdrow"], dtype=np.float64).sum(axis=0)
        partition += (np.log(pd) + 511.0 * GAMMA).sum()
        gold += np.asarray(r["gold"], dtype=np.float64).sum()
    out = np.float32(partition - gold)
    if _trace:
        return out, res
    return out


# revision 15
# speedup vs baseline: 1.0116x; 1.0007x over previous
"""CRF negative-log-likelihood kernel for Trainium2 (8 NeuronCores).

Math: reference computes  partition - gold  where
  partition = sum_b logsumexp_c(alpha[511])  via the forward algorithm
  gold      = sum emissions[b,s,tags] * m + sum T[tags[s],tags[s+1]] * m[:,1:]

Device strategy (data-parallel over batch, 32 rows per core):
  * Linear domain: alpha_t = E_t o (A' @ alpha_{t-1}) with A' = exp(T) e^-g,
    E_t = exp(e_t).  The per-step logsumexp becomes a [128,128]x[128,32]
    matmul (PE) plus an elementwise multiply (DVE).
  * Bidirectional scan, PAIRED: forward (A'^T) and backward (A') states
    live in one [128,64] tile [alphaF | vB]; the host lays emissions out
    so pair-step k holds [E_k | E_{511-k}].  Per step: 2 matmuls into one
    PSUM tile + ONE DVE multiply.  256 serial steps instead of 511.
  * Stability WITHOUT renorm: the constant growth rate g=GAMMA (calibrated
    offline; per-step ln growth of the scan state) is folded into the
    transition matrix on the host.  State magnitudes then do a bounded
    random walk (~2^+-14, far inside bf16 range) and the host adds the
    exact known correction 511*GAMMA per batch element.  This keeps the
    scan loop free of colsum/reciprocal/broadcast work, so PE and DVE run
    nothing but the serial chain.
  * Gold emit: masked sum e o onehot(tags) chunk-wise entirely on GPSIMD
    (Pool) via fused scalar_tensor_tensor with accum_out; Pool is off the
    scan critical path, so no anchoring is needed.
  * Gold trans: exact masked pair-count matrix CNT (host-built, index-only
    preprocessing) dotted with T on Pool at the end.
Outputs per core: meeting-dot row pdrow, gold partials; host sums in
float64, adds 511*GAMMA per batch element, returns a float32 scalar.
"""

import sys

for _p in ("/opt/trn_rl_repo",):
    if _p not in sys.path:
        sys.path.insert(0, _p)

import os as _os
import numpy as np
import ml_dtypes
from contextlib import ExitStack

from concourse import bass, tile, mybir, bacc
from concourse.bass_utils import run_bass_kernel_spmd

NCORES = 8
B, S, C = 256, 512, 128
BC = B // NCORES          # batch rows per core
FREE = S * BC             # free-dim elements of the per-core emission tensor
PAIRW = 2 * BC            # 64: [E_k | E_{S-1-k}]
HALF = S // 2             # pair-steps: fwd e_0..e_255, bwd e_256..e_511

# calibrated mean per-step ln growth of the paired scan state (see
# calibrate.py); folded into the transition matrix as exp(-GAMMA) and
# compensated exactly on the host with +511*GAMMA per batch element.
GAMMA = 5.8644

# emission chunk sizes (free elements); small leading chunks let the scan
# chain start before the bulk DMA+exp completes
CH_SIZES = [320, 704, 1024] + [2048] * 7
CH_OFF = [0]
for _s in CH_SIZES:
    CH_OFF.append(CH_OFF[-1] + _s)
assert CH_OFF[-1] == FREE
NCHUNK = len(CH_SIZES)

F32 = mybir.dt.float32
BF16 = mybir.dt.bfloat16
AF = mybir.ActivationFunctionType
OP = mybir.AluOpType

_EN_GOLD = _os.environ.get("CRF_GOLD", "1") == "1"
_EN_SCAN = _os.environ.get("CRF_SCAN", "1") == "1"

_NC_CACHE = None


def _build_nc():
    nc = bacc.Bacc("TRN2", target_bir_lowering=False, debug=False)

    et = nc.dram_tensor("et", [C, FREE], BF16, kind="ExternalInput").ap()
    afwd = nc.dram_tensor("afwd", [C, C], BF16, kind="ExternalInput").ap()
    abwd = nc.dram_tensor("abwd", [C, C], BF16, kind="ExternalInput").ap()
    hemit = nc.dram_tensor("hemit", [C, FREE], BF16, kind="ExternalInput").ap()
    cnt_in = nc.dram_tensor("cnt", [C, C], F32, kind="ExternalInput").ap()
    tsb_in = nc.dram_tensor("tsb", [C, C], F32, kind="ExternalInput").ap()
    pdrow = nc.dram_tensor("pdrow", [C, BC], F32, kind="ExternalOutput").ap()
    gold = nc.dram_tensor("gold", [128, 1], F32, kind="ExternalOutput").ap()

    with tile.TileContext(nc) as tc, ExitStack() as ctx:
        sb = ctx.enter_context(tc.tile_pool(name="sb", bufs=1))
        wk = ctx.enter_context(tc.tile_pool(name="wk", bufs=4))
        ps = ctx.enter_context(tc.tile_pool(name="ps", bufs=2, space="PSUM"))

        # ---- persistent tiles; chunk0 DMA first (longest startup path) --
        wf = sb.tile([C, C], BF16, name="wf")
        wb_ = sb.tile([C, C], BF16, name="wb")

        # ---- emission chunks: DMA in + exp ------------------------------
        raws, ecs = [], []
        et_dmas = []
        for k, csz in enumerate(CH_SIZES):
            raw = sb.tile([C, csz], BF16, name=f"raw{k}")
            et_dmas.append(
                nc.sync.dma_start(raw[:], et[:, CH_OFF[k]:CH_OFF[k] + csz]))
            raws.append(raw)
            ec = sb.tile([C, csz], BF16, name=f"ec{k}")
            ecs.append(ec)
            if k == 0:
                nc.sync.dma_start(wf[:], afwd[:])
                nc.sync.dma_start(wb_[:], abwd[:])

        NEARLY = 2            # chunks whose exp runs before the scan starts
        def exp_chunk(c):
            nc.scalar.activation(ecs[c][:], raws[c][:], AF.Exp)
        for c in range(NEARLY):
            exp_chunk(c)

        def ec_pair(k, lo=0, hi=PAIRW):
            pos = k * PAIRW
            for c in range(NCHUNK):
                if pos < CH_OFF[c + 1]:
                    o = pos - CH_OFF[c]
                    return ecs[c][:, o + lo:o + hi]
            raise IndexError(k)

        # ---- gold: all on Pool (off the scan critical path) -------------
        from concourse.tile_rust import add_dep_helper
        gold_finish = None
        if not _EN_GOLD:
            zg = sb.tile([128, 1], F32, name="zg")
            nc.vector.memset(zg[:], 0.0)
            nc.sync.dma_start(gold[:], zg[:])
        if not _EN_SCAN:
            zl = sb.tile([C, BC], F32, name="zl")
            nc.vector.memset(zl[:], 1.0)
            nc.sync.dma_start(pdrow[:], zl[:])

        if _EN_GOLD:
            hem_sb = sb.tile([C, FREE], BF16, name="hem_sb")
            cnt_sb = sb.tile([C, C], F32, name="cnt_sb")
            tsb = sb.tile([C, C], F32, name="tsb_t")
            last_et = et_dmas[-1].ins
            qs = FREE // 8
            for k in range(8):
                gd = nc.sync.dma_start(hem_sb[:, k * qs:(k + 1) * qs],
                                       hemit[:, k * qs:(k + 1) * qs])
                add_dep_helper(gd.ins, last_et,
                               reason="gold DMA after emission stream")
            for gd in (nc.sync.dma_start(cnt_sb[:], cnt_in[:]),
                       nc.sync.dma_start(tsb[:], tsb_in[:])):
                add_dep_helper(gd.ins, last_et,
                               reason="gold DMA after emission stream")

            gold_acc = sb.tile([128, 1], F32, name="gold_acc")
            nc.gpsimd.memset(gold_acc[:], 0.0)

            # emit pieces: fused multiply+row-sum on Pool
            pieces = []
            for c, csz in enumerate(CH_SIZES):
                o = 0
                while o < csz:
                    w = min(512, csz - o)
                    pieces.append((c, o, w))
                    o += w

            def emit_piece(j):
                c, o, w = pieces[j]
                scratch = wk.tile([C, 512], BF16, tag="scr", bufs=2,
                                  name=f"scr{j}")
                epk = wk.tile([128, 1], F32, tag="ep", bufs=2, name=f"ep{j}")
                nc.gpsimd.tensor_mul(
                    scratch[:, 0:w], raws[c][:, o:o + w],
                    hem_sb[:, CH_OFF[c] + o:CH_OFF[c] + o + w])
                nc.scalar.activation(scratch[:, 0:w], scratch[:, 0:w],
                                     AF.Identity, accum_out=epk[:])
                nc.gpsimd.tensor_add(gold_acc[:], gold_acc[:], epk[:])

            def gold_finish():
                trash = sb.tile([128, 128], F32, name="trash")
                tp = sb.tile([128, 1], F32, name="tp")
                nc.gpsimd.tensor_mul(trash[:], cnt_sb[:], tsb[:])
                nc.scalar.activation(trash[:], trash[:], AF.Identity,
                                     accum_out=tp[:])
                gold_sb = sb.tile([128, 1], F32, name="gold_sb")
                nc.gpsimd.tensor_add(gold_sb[:], gold_acc[:], tp[:])
                nc.sync.dma_start(gold[:], gold_sb[:])

        # injection schedule: value = list of zero-arg callables issued
        # after scan step k.  This controls per-engine FIFO order only:
        # exp chunks must reach the Activation queue ahead of emit-accum
        # pieces that block on the (late) hemit DMA stream.
        #
        # ec prefetch: the first DVE reader of a fresh ec chunk carries an
        # extra Activation wait; Tile then parks the chunk-boundary TT's
        # matmul wait on a blocking SEQ-level EventSemaphore, which stalls
        # TT dispatch for ~200ns.  A dummy 1-column read of each chunk a
        # few steps early absorbs the Act wait off the critical path.
        def prefetch_ec(c):
            dum = wk.tile([C, 1], BF16, tag="dum", bufs=2, name=f"dum{c}")
            nc.vector.tensor_copy(dum[:], ecs[c][:, 0:1])

        inject_at = {}
        if _EN_SCAN:
            exp_step = {}
            for c in range(NEARLY, NCHUNK):
                k_need = CH_OFF[c] // PAIRW
                lead = 8 if c < 3 else 20
                exp_step[c] = max(2, k_need - lead)
                inject_at.setdefault(exp_step[c], []).append(
                    lambda c=c: exp_chunk(c))
            for c in range(1, NCHUNK):
                k_need = CH_OFF[c] // PAIRW
                ds = max(exp_step.get(c, 0) + 4, k_need - 6, 3)
                inject_at.setdefault(min(ds, k_need - 1), []).append(
                    lambda c=c: prefetch_ec(c))
            if _EN_GOLD:
                for j in range(len(pieces)):
                    inject_at.setdefault(40 + 6 * j, []).append(
                        lambda j=j: emit_piece(j))
        else:
            for c in range(NEARLY, NCHUNK):
                exp_chunk(c)
            if _EN_GOLD:
                for j in range(len(pieces)):
                    emit_piece(j)

        if _EN_SCAN:
            # ---- bidirectional scan, SPLIT chains: the forward and ------
            # backward states are independent serial chains (they only meet
            # at the end), so each runs its own MM -> TT cycle; the two
            # cycles interleave on PE/DVE, halving per-instruction exec on
            # the critical path.
            aF = ec_pair(0, 0, BC)          # E_0
            vB = ec_pair(0, BC, PAIRW)      # E_511
            for k in range(1, HALF):
                ppF = ps.tile([C, BC], F32, tag="ppF", bufs=3, name=f"pf{k}")
                nc.tensor.matmul(ppF[:], wf[:], aF, start=True, stop=True)
                aF_new = wk.tile([C, BC], BF16, tag="aF", bufs=6,
                                 name=f"aF{k}")
                nc.vector.tensor_tensor(aF_new[:], ppF[:], ec_pair(k, 0, BC),
                                        op=OP.mult)
                aF = aF_new[:]

                ppB = ps.tile([C, BC], F32, tag="ppB", bufs=3, name=f"pb{k}")
                nc.tensor.matmul(ppB[:], wb_[:], vB, start=True, stop=True)
                vB_new = wk.tile([C, BC], BF16, tag="vB", bufs=6,
                                 name=f"vB{k}")
                nc.vector.tensor_tensor(vB_new[:], ppB[:],
                                        ec_pair(k, BC, PAIRW), op=OP.mult)
                vB = vB_new[:]
                for job in inject_at.get(k, []):
                    job()

            # ---- combine: d[c,b] = alphaF[c,b] * (A' vB)[c,b]; the ------
            # column sum and log run on the host (shorter device tail)
            pbf = ps.tile([C, BC], F32, tag="ppB", bufs=3, name="pb_final")
            nc.tensor.matmul(pbf[:], wb_[:], vB, start=True, stop=True)
            d = wk.tile([C, BC], F32, tag="dm", bufs=1, name="d_meet")
            nc.vector.tensor_tensor(d[:], pbf[:], aF, op=OP.mult)
            nc.sync.dma_start(pdrow[:], d[:])
        if _EN_GOLD:
            gold_finish()

    nc.compile()
    return nc


def _prep_inputs(emissions, tags, mask, transitions):
    em = np.asarray(emissions, dtype=np.float32)
    tg = np.asarray(tags).astype(np.int64)
    mk = np.asarray(mask).astype(np.float32)
    tr = np.ascontiguousarray(np.asarray(transitions, dtype=np.float32))

    a_f = np.exp(tr.astype(np.float64) - GAMMA)
    afwd = a_f.astype(ml_dtypes.bfloat16)
    abwd = np.ascontiguousarray(a_f.T).astype(ml_dtypes.bfloat16)

    # paired free layout: pair-step k holds [E_k | E_{S-1-k}] in 64 cols
    s_all = np.arange(S, dtype=np.int64)
    pair_base = np.where(s_all < S // 2, s_all * PAIRW,
                         (S - 1 - s_all) * PAIRW + BC)   # [S]
    b_rows = np.arange(BC, dtype=np.int64)[:, None]      # [BC,1]
    sbcol = (pair_base[None, :] + b_rows).ravel()        # free idx for (b,s)

    in_maps = []
    for core in range(NCORES):
        b0 = core * BC
        ec = em[b0:b0 + BC]                              # [BC,S,C]
        ett = ec.transpose(2, 1, 0)                      # [C,S,BC]
        half = S // 2
        et = np.empty((C, half, PAIRW), dtype=np.float32)
        et[:, :, :BC] = ett[:, :half, :]                 # fwd slot: E_k
        et[:, :, BC:] = ett[:, :half - 1:-1, :]          # bwd slot: E_{S-1-k}
        et = np.ascontiguousarray(
            et.reshape(C, FREE)).astype(ml_dtypes.bfloat16)

        tgc = tg[b0:b0 + BC]                             # [BC,S]
        mkc = mk[b0:b0 + BC]

        hemit = np.zeros((C, FREE), dtype=ml_dtypes.bfloat16)
        hemit[tgc.ravel(), sbcol] = mkc.ravel()

        # masked pair-count histogram (index-only preprocessing; the
        # float gather-sum  sum T[i,j]*CNT[i,j]  runs on device)
        cnt = np.zeros((C, C), dtype=np.float64)
        np.add.at(cnt, (tgc[:, :-1].ravel(), tgc[:, 1:].ravel()),
                  mkc[:, 1:].ravel().astype(np.float64))
        cnt = cnt.astype(np.float32)

        in_maps.append({
            "et": et, "afwd": afwd, "abwd": abwd,
            "hemit": hemit, "cnt": cnt, "tsb": tr,
        })
    return in_maps


def kernel(emissions, tags, mask, transitions, _trace=False):
    global _NC_CACHE
    if _NC_CACHE is None:
        _NC_CACHE = _build_nc()
    nc = _NC_CACHE

    in_maps = _prep_inputs(emissions, tags, mask, transitions)
    res = run_bass_kernel_spmd(
        nc, in_maps, core_ids=list(range(NCORES)), trace=_trace,
    )
    partition = np.float64(0.0)
    gold = np.float64(0.0)
    for r in res.results:
        pd = np.asarray(r["pdrow"], dtype=np.float64).sum(axis=0)
        partition += (np.log(pd) + 511.0 * GAMMA).sum()
        gold += np.asarray(r["gold"], dtype=np.float64).sum()
    out = np.float32(partition - gold)
    if _trace:
        return out, res
    return out
